# revision 18
# baseline (speedup 1.0000x reference)
"""Fused multi-head self-attention kernel for Trainium2 (8 NeuronCores).

Problem: B=4, L=2048, H=8, DK=DV=64 (fp32), softmax(QK^T * sqrt(64)) V, then
output projection with a dv-major head flatten.

Sharding: pure data-parallel over (batch, query-half): core c handles batch
c//2 and query rows [half*1024, (half+1)*1024) where half = c%2.  Each core
receives its batch's x rows rotated so its query rows come first (softmax over
keys is permutation invariant), computes K/V for all 2048 keys and Q for its
1024 queries, and writes a disjoint [1024, 64] output slice.  No collectives.

On-device layout (per core):
  xT   [64, 2048]   x^T via PE transposes
  KT_h [64, 2048]   (x @ wk + bk)^T per head
  QT_h [64, 1024]   same for queries
  V_h  [2048, 65]   x @ wv + bv per head, chunked [128, 65] with a ones
                    column -> the PV matmul also accumulates the softmax
                    denominator as row 64 of its output.
  Attention per head: S^T chunks [k=128, q=1024] on PE -> exp(8*S - 70) on
  ACT (C=70 is a global stability offset; it cancels in the softmax ratio)
  -> PV matmul accumulates ctx^T [65, 1024] over the 16 k-chunks.
  Normalize by the denominator row, assemble ctx^T chunks h-major, and apply
  the output projection with a host-permuted wo (h-major rows), then + bo.
"""

import sys

sys.path.insert(0, "/opt/trn_rl_repo")

import numpy as np

import concourse.bass as bass
import concourse.mybir as mybir
import concourse.tile as tile
from concourse import bacc
from concourse.bass_utils import run_bass_kernel_spmd
from concourse.masks import make_identity

B, L, H, D = 4, 2048, 8, 64
LQ = L // 2            # queries per core
NKC = L // 128         # 16 key chunks
F32 = mybir.dt.float32
F16 = mybir.dt.float16     # QK path: 11-bit mantissa, 1 cyc/row, FWL
BF16 = mybir.dt.bfloat16   # PV path: exp() needs the fp32 exponent range
C_OFF = 70.0           # softmax stability offset, cancels in the ratio

TRACE = False          # set by test.py for profiled runs
TRACE_DIR = None       # optional dir to keep NTFF/perfetto artifacts
LAST_RESULTS = None

_cache = {}


def _emit(nc, tc, ctx):
    xin = nc.dram_tensor("xin", [L, D], F16, kind="ExternalInput").ap()
    wq_d = nc.dram_tensor("wq", [D, H * D], F16, kind="ExternalInput").ap()
    wk_d = nc.dram_tensor("wk", [D, H * D], F16, kind="ExternalInput").ap()
    wv_d = nc.dram_tensor("wv", [D, H * D], F16, kind="ExternalInput").ap()
    bq_d = nc.dram_tensor("bq", [H * D], F32, kind="ExternalInput").ap()
    bk_d = nc.dram_tensor("bk", [H * D], F32, kind="ExternalInput").ap()
    bv_d = nc.dram_tensor("bv", [H * D], F32, kind="ExternalInput").ap()
    wo_d = nc.dram_tensor("wo_p", [H * D, D], F16, kind="ExternalInput").ap()
    bo_d = nc.dram_tensor("bo", [D], F32, kind="ExternalInput").ap()
    o_d = nc.dram_tensor("o", [LQ, D], F32, kind="ExternalOutput").ap()

    singles = ctx.enter_context(tc.tile_pool(name="singles", bufs=1))

    # --- constants / weights in SBUF ---
    wq_sb = singles.tile([D, H * D], F16, name="wq_sb")
    nc.sync.dma_start(out=wq_sb, in_=wq_d)
    wk_sb = singles.tile([D, H * D], F16, name="wk_sb")
    nc.sync.dma_start(out=wk_sb, in_=wk_d)
    wv_sb = singles.tile([D, H * D], F16, name="wv_sb")
    nc.sync.dma_start(out=wv_sb, in_=wv_d)
    # wo_p rows are h-major: row h*64+dv.  Partition chunks of 128.
    wo_sb = singles.tile([128, 4, D], F16, name="wo_sb")
    nc.sync.dma_start(out=wo_sb, in_=wo_d.rearrange("(j p) e -> p j e", p=128))
    # per-partition bias columns for the Q/K projections: bqT[d, h]
    bqT = singles.tile([D, H], F32, name="bqT")
    nc.sync.dma_start(out=bqT, in_=bq_d.rearrange("(h d) -> d h", h=H))
    bkT = singles.tile([D, H], F32, name="bkT")
    nc.sync.dma_start(out=bkT, in_=bk_d.rearrange("(h d) -> d h", h=H))
    bv_bc = singles.tile([128, H * D], F32, name="bv_bc")
    nc.sync.dma_start(out=bv_bc, in_=bv_d.rearrange("(h e) -> h e", h=1).to_broadcast([128, H * D]))
    bo_bc = singles.tile([128, D], F32, name="bo_bc")
    nc.sync.dma_start(out=bo_bc, in_=bo_d.rearrange("(h e) -> h e", h=1).to_broadcast([128, D]))

    # --- persistent SBUF tensors ---
    xT = singles.tile([D, L], F16, name="xT")
    kt = [singles.tile([D, L], F16, name=f"kt{h}") for h in range(H)]
    qt = [singles.tile([D, LQ], F16, name=f"qt{h}") for h in range(H)]
    # V packed [128, kc, h, 65]; column 64 is the ones column.
    v_all = singles.tile([128, NKC * H * 65], BF16, name="v_all")
    v_view = v_all.rearrange("p (kc h e) -> p kc h e", kc=NKC, h=H)
    ct = [singles.tile([128, LQ], F16, name=f"ct{j}") for j in range(4)]

    ones_f32 = singles.tile([128, NKC * H], F32, name="ones_f32")
    nc.vector.memset(ones_f32, 1.0)
    nc.vector.tensor_copy(
        v_view[:, :, :, 64:65],
        ones_f32.rearrange("p (a b o) -> p a b o", a=NKC, o=1))

    neg_c = singles.tile([128, 1], F32, name="neg_c")
    nc.vector.memset(neg_c, -C_OFF)

    # ---------------- Phase A: x -> xT via PE transposes ----------------
    with tc.tile_pool(name="xload", bufs=3) as xpool, \
         tc.tile_pool(name="trp", bufs=3, space="PSUM") as trpool:
        identity = xpool.tile([128, 128], F16, name="identity", tag="ident")
        make_identity(nc, identity)
        for t in range(L // 128):
            xt_in = xpool.tile([128, D], F16, tag="xt_in")
            nc.sync.dma_start(out=xt_in, in_=xin[t * 128:(t + 1) * 128, :])
            trp = trpool.tile([D, 128], F16, tag="trp")
            nc.tensor.transpose(trp, xt_in, identity)
            nc.vector.tensor_copy(xT[:, t * 128:(t + 1) * 128], trp)

    # ------- Phases B+C fused: V + per-head projections + attention -------
    # PE instruction order is the schedule (engines issue in order): for each
    # head, ST(kc+1) is emitted before PV(kc) so the PE never sits behind a
    # wait on the exp it just enabled; the next head's K/Q projections fill
    # the pipeline-drain window at the head boundary.
    stpool = ctx.enter_context(tc.tile_pool(name="stp", bufs=3, space="PSUM"))
    pvpool = ctx.enter_context(tc.tile_pool(name="pvp", bufs=1, space="PSUM"))
    ptpool = ctx.enter_context(tc.tile_pool(name="ptp", bufs=2))
    denpool = ctx.enter_context(tc.tile_pool(name="denp", bufs=2))
    dendram = ctx.enter_context(tc.tile_pool(name="dendram", bufs=2, space="DRAM"))

    def emit_v(lc):
        pv_ = stpool.tile([128, LQ], F32, tag="st", name=f"vp{lc}")
        nc.tensor.matmul(
            pv_[:, 0:512], xT[:, lc * 128:(lc + 1) * 128], wv_sb,
            start=True, stop=True)
        nc.vector.tensor_add(
            v_view[:, lc, :, 0:64],
            pv_[:, 0:512].rearrange("p (h e) -> p h e", h=H),
            bv_bc.rearrange("p (h e) -> p h e", h=H))

    def emit_proj(h):
        for ks in range(L // 512):
            pk = stpool.tile([128, LQ], F32, tag="st", name=f"pk{h}_{ks}")
            nc.tensor.matmul(
                pk[0:64, 0:512], wk_sb[:, h * D:(h + 1) * D],
                xT[:, ks * 512:(ks + 1) * 512],
                start=True, stop=True)
            nc.vector.tensor_scalar_add(
                kt[h][:, ks * 512:(ks + 1) * 512], pk[0:64, 0:512],
                bkT[:, h:h + 1])
        for qs in range(LQ // 512):
            pq = stpool.tile([128, LQ], F32, tag="st", name=f"pq{h}_{qs}")
            nc.tensor.matmul(
                pq[0:64, 0:512], wq_sb[:, h * D:(h + 1) * D],
                xT[:, qs * 512:(qs + 1) * 512],
                start=True, stop=True)
            nc.vector.tensor_scalar_add(
                qt[h][:, qs * 512:(qs + 1) * 512], pq[0:64, 0:512],
                bqT[:, h:h + 1])

    for lc in range(NKC):
        emit_v(lc)
    emit_proj(0)

    for h in range(H):
        j, half = h // 2, h % 2
        pv = pvpool.tile([65, LQ], F32, tag="pv")
        pts = {}

        def emit_pv(kc):
            for qs in range(LQ // 512):
                nc.tensor.matmul(
                    pv[:, qs * 512:(qs + 1) * 512],
                    v_view[:, kc, h, :],
                    pts[kc][:, qs * 512:(qs + 1) * 512],
                    start=(kc == 0), stop=(kc == NKC - 1))
            del pts[kc]

        for kc in range(NKC):
            st = stpool.tile([128, LQ], F32, tag="st")
            for qs in range(LQ // 512):
                nc.tensor.matmul(
                    st[:, qs * 512:(qs + 1) * 512],
                    kt[h][:, kc * 128:(kc + 1) * 128],
                    qt[h][:, qs * 512:(qs + 1) * 512],
                    start=True, stop=True)
            if kc > 0:
                emit_pv(kc - 1)
            pt = ptpool.tile([128, LQ], BF16, tag="pt")
            nc.scalar.activation(pt, st, mybir.ActivationFunctionType.Exp,
                                 bias=neg_c, scale=8.0)
            pts[kc] = pt
        if h + 1 < H:
            emit_proj(h + 1)   # fills the PE while exp(15) completes
        emit_pv(NKC - 1)

        # normalize: den row -> DRAM bounce -> [64,16] exact reciprocal ->
        # DRAM -> partition-broadcast [64, LQ]
        den_sb = denpool.tile([1, LQ], F32, tag="den_sb")
        nc.vector.tensor_copy(den_sb, pv[64:65, :])
        den_d = dendram.tile([1, LQ], F32, tag="den_d")
        nc.sync.dma_start(out=den_d, in_=den_sb)
        den64 = denpool.tile([64, 16], F32, tag="den64")
        nc.sync.dma_start(out=den64,
                          in_=den_d.rearrange("o (p i) -> (o p) i", p=64))
        rec64 = denpool.tile([64, 16], F32, tag="rec64")
        nc.vector.reciprocal(rec64, den64)
        rec_d = dendram.tile([1, LQ], F32, tag="rec_d")
        nc.sync.dma_start(out=rec_d.rearrange("o (p i) -> (o p) i", p=64),
                          in_=rec64)
        den_b = denpool.tile([64, LQ], F32, tag="den_b")
        nc.sync.dma_start(out=den_b, in_=rec_d.to_broadcast([64, LQ]))
        nc.vector.tensor_mul(
            ct[j][half * 64:(half + 1) * 64, :], pv[0:64, :], den_b)

    # ---------------- Phase D: output projection ----------------
    with tc.tile_pool(name="outs", bufs=3) as outsb:
        for lc in range(LQ // 128):
            pot = stpool.tile([128, LQ], F32, tag="st", name=f"po{lc}")
            po = pot[:, 0:D]
            for j in range(4):
                nc.tensor.matmul(
                    po, ct[j][:, lc * 128:(lc + 1) * 128], wo_sb[:, j, :],
                    start=(j == 0), stop=(j == 3))
            osb = outsb.tile([128, D], F32, tag="osb")
            nc.vector.tensor_add(osb, po, bo_bc)
            nc.sync.dma_start(out=o_d[lc * 128:(lc + 1) * 128, :], in_=osb)


def _build():
    if "nc" in _cache:
        return _cache["nc"]
    from contextlib import ExitStack
    nc = bacc.Bacc("TRN2", target_bir_lowering=False, debug=False)
    with tile.TileContext(nc) as tc:
        with ExitStack() as ctx:
            _emit(nc, tc, ctx)
    nc.finalize()
    _cache["nc"] = nc
    return nc


def kernel(x, wq, bq, wk, bk, wv, bv, wo, bo):
    global LAST_RESULTS
    x = np.ascontiguousarray(np.asarray(x, dtype=np.float32))
    wq = np.ascontiguousarray(np.asarray(wq, dtype=np.float32))
    wk = np.ascontiguousarray(np.asarray(wk, dtype=np.float32))
    wv = np.ascontiguousarray(np.asarray(wv, dtype=np.float32))
    wo = np.ascontiguousarray(np.asarray(wo, dtype=np.float32))
    bq = np.ascontiguousarray(np.asarray(bq, dtype=np.float32))
    bk = np.ascontiguousarray(np.asarray(bk, dtype=np.float32))
    bv = np.ascontiguousarray(np.asarray(bv, dtype=np.float32))
    bo = np.ascontiguousarray(np.asarray(bo, dtype=np.float32))

    # wo rows are dv-major (row = dv*H + h); permute to h-major (row = h*D+dv)
    wo_p = np.ascontiguousarray(
        wo.reshape(D, H, D).transpose(1, 0, 2).reshape(H * D, D))
    # QK/PV matmul operands are fed in 16-bit; convert weights on host
    wq16 = wq.astype(np.float16)
    wk16 = wk.astype(np.float16)
    wv16 = wv.astype(np.float16)
    wo16 = wo_p.astype(np.float16)

    nc = _build()
    in_maps = []
    for c in range(8):
        b, half = c // 2, c % 2
        xb = x[b]
        if half:
            xb = np.concatenate([xb[LQ:], xb[:LQ]], axis=0)
        xb = np.ascontiguousarray(xb.astype(np.float16))
        in_maps.append({
            "xin": xb, "wq": wq16, "wk": wk16, "wv": wv16, "wo_p": wo16,
            "bq": bq, "bk": bk, "bv": bv, "bo": bo,
        })
    res = run_bass_kernel_spmd(nc, in_maps, core_ids=list(range(8)),
                               trace=TRACE, tmpdir=TRACE_DIR)
    LAST_RESULTS = res
    out = np.empty((B, L, D), dtype=np.float32)
    for c in range(8):
        b, half = c // 2, c % 2
        out[b, half * LQ:(half + 1) * LQ, :] = res.results[c]["o"]
    return out


# revision 19
# speedup vs baseline: 1.1171x; 1.1171x over previous
"""Fused multi-head self-attention kernel for Trainium2 (8 NeuronCores).

Problem: B=4, L=2048, H=8, DK=DV=64 (fp32), softmax(QK^T * sqrt(64)) V, then
output projection with a dv-major head flatten.

Sharding: pure data-parallel over (batch, query-half): core c handles batch
c//2 and query rows [half*1024, (half+1)*1024) where half = c%2.  Each core
receives its batch's x rows rotated so its query rows come first (softmax over
keys is permutation invariant), computes K/V for all 2048 keys and Q for its
1024 queries, and writes a disjoint [1024, 64] output slice.  No collectives.

On-device layout (per core):
  xT   [64, 2048]   x^T via PE transposes
  KT_h [64, 2048]   (x @ wk + bk)^T per head
  QT_h [64, 1024]   same for queries
  V_h  [2048, 65]   x @ wv + bv per head, chunked [128, 65] with a ones
                    column -> the PV matmul also accumulates the softmax
                    denominator as row 64 of its output.
  Attention per head: S^T chunks [k=128, q=1024] on PE -> exp(8*S - 70) on
  ACT (C=70 is a global stability offset; it cancels in the softmax ratio)
  -> PV matmul accumulates ctx^T [65, 1024] over the 16 k-chunks.
  Normalize by the denominator row, assemble ctx^T chunks h-major, and apply
  the output projection with a host-permuted wo (h-major rows), then + bo.
"""

import sys

sys.path.insert(0, "/opt/trn_rl_repo")

import numpy as np

import concourse.bass as bass
import concourse.mybir as mybir
import concourse.tile as tile
from concourse import bacc
from concourse.bass_utils import run_bass_kernel_spmd
from concourse.masks import make_identity

B, L, H, D = 4, 2048, 8, 64
LQ = L // 2            # queries per core
NKC = L // 128         # 16 key chunks
F32 = mybir.dt.float32
F16 = mybir.dt.float16     # QK path: 11-bit mantissa, 1 cyc/row, FWL
BF16 = mybir.dt.bfloat16   # PV path: exp() needs the fp32 exponent range
C_OFF = 70.0           # softmax stability offset, cancels in the ratio

TRACE = False          # set by test.py for profiled runs
TRACE_DIR = None       # optional dir to keep NTFF/perfetto artifacts
LAST_RESULTS = None

_cache = {}


def _emit(nc, tc, ctx):
    xin = nc.dram_tensor("xin", [L, D], F16, kind="ExternalInput").ap()
    wq_d = nc.dram_tensor("wq", [D, H * D], F16, kind="ExternalInput").ap()
    wk_d = nc.dram_tensor("wk", [D, H * D], F16, kind="ExternalInput").ap()
    wv_d = nc.dram_tensor("wv", [D, H * D], F16, kind="ExternalInput").ap()
    bq_d = nc.dram_tensor("bq", [H * D], F32, kind="ExternalInput").ap()
    bk_d = nc.dram_tensor("bk", [H * D], F32, kind="ExternalInput").ap()
    bv_d = nc.dram_tensor("bv", [H * D], F32, kind="ExternalInput").ap()
    wo_d = nc.dram_tensor("wo_p", [H * D, D], F16, kind="ExternalInput").ap()
    bo_d = nc.dram_tensor("bo", [D], F32, kind="ExternalInput").ap()
    o_d = nc.dram_tensor("o", [LQ, D], F32, kind="ExternalOutput").ap()

    singles = ctx.enter_context(tc.tile_pool(name="singles", bufs=1))

    # --- constants / weights in SBUF ---
    wq_sb = singles.tile([D, H * D], F16, name="wq_sb")
    nc.sync.dma_start(out=wq_sb, in_=wq_d)
    wk_sb = singles.tile([D, H * D], F16, name="wk_sb")
    nc.sync.dma_start(out=wk_sb, in_=wk_d)
    wv_sb = singles.tile([D, H * D], F16, name="wv_sb")
    nc.sync.dma_start(out=wv_sb, in_=wv_d)
    # wo_p rows are h-major: row h*64+dv.  Partition chunks of 128.
    wo_sb = singles.tile([128, 4, D], F16, name="wo_sb")
    nc.sync.dma_start(out=wo_sb, in_=wo_d.rearrange("(j p) e -> p j e", p=128))
    # per-partition bias columns for the Q/K projections: bqT[d, h]
    bqT = singles.tile([D, H], F32, name="bqT")
    nc.sync.dma_start(out=bqT, in_=bq_d.rearrange("(h d) -> d h", h=H))
    bkT = singles.tile([D, H], F32, name="bkT")
    nc.sync.dma_start(out=bkT, in_=bk_d.rearrange("(h d) -> d h", h=H))
    bv_bc = singles.tile([128, H * D], F32, name="bv_bc")
    nc.sync.dma_start(out=bv_bc, in_=bv_d.rearrange("(h e) -> h e", h=1).to_broadcast([128, H * D]))
    bo_bc = singles.tile([128, D], F32, name="bo_bc")
    nc.sync.dma_start(out=bo_bc, in_=bo_d.rearrange("(h e) -> h e", h=1).to_broadcast([128, D]))

    # --- persistent SBUF tensors ---
    xT = singles.tile([D, L], F16, name="xT")
    kt = [singles.tile([D, L], F16, name=f"kt{h}") for h in range(H)]
    qt = [singles.tile([D, LQ], F16, name=f"qt{h}") for h in range(H)]
    # V packed [128, kc, h, 65]; column 64 is the ones column.
    v_all = singles.tile([128, NKC * H * 65], BF16, name="v_all")
    v_view = v_all.rearrange("p (kc h e) -> p kc h e", kc=NKC, h=H)
    ct = [singles.tile([128, LQ], F16, name=f"ct{j}") for j in range(4)]

    ones_f32 = singles.tile([128, NKC * H], F32, name="ones_f32")
    nc.vector.memset(ones_f32, 1.0)
    nc.vector.tensor_copy(
        v_view[:, :, :, 64:65],
        ones_f32.rearrange("p (a b o) -> p a b o", a=NKC, o=1))

    neg_c = singles.tile([128, 1], F32, name="neg_c")
    nc.vector.memset(neg_c, -C_OFF)

    # ---------------- Phase A: x -> xT via PE transposes ----------------
    with tc.tile_pool(name="xload", bufs=3) as xpool, \
         tc.tile_pool(name="trp", bufs=3, space="PSUM") as trpool:
        identity = xpool.tile([128, 128], F16, name="identity", tag="ident")
        make_identity(nc, identity)
        for t in range(L // 128):
            xt_in = xpool.tile([128, D], F16, tag="xt_in")
            nc.sync.dma_start(out=xt_in, in_=xin[t * 128:(t + 1) * 128, :])
            trp = trpool.tile([D, 128], F16, tag="trp")
            nc.tensor.transpose(trp, xt_in, identity)
            nc.vector.tensor_copy(xT[:, t * 128:(t + 1) * 128], trp)

    # ------- Phases B+C fused: V + per-head projections + attention -------
    # PE instruction order is the schedule (engines issue in order): for each
    # head, ST(kc+1) is emitted before PV(kc) so the PE never sits behind a
    # wait on the exp it just enabled; the next head's K/Q projections fill
    # the pipeline-drain window at the head boundary.
    stpool = ctx.enter_context(tc.tile_pool(name="stp", bufs=2, space="PSUM"))
    pvpool = ctx.enter_context(tc.tile_pool(name="pvp", bufs=2, space="PSUM"))
    ptpool = ctx.enter_context(tc.tile_pool(name="ptp", bufs=3))
    denpool = ctx.enter_context(tc.tile_pool(name="denp", bufs=2))
    dendram = ctx.enter_context(tc.tile_pool(name="dendram", bufs=2, space="DRAM"))

    def emit_v(lc):
        pv_ = stpool.tile([128, LQ], F32, tag="st", name=f"vp{lc}")
        nc.tensor.matmul(
            pv_[:, 0:512], xT[:, lc * 128:(lc + 1) * 128], wv_sb,
            start=True, stop=True)
        nc.vector.tensor_add(
            v_view[:, lc, :, 0:64],
            pv_[:, 0:512].rearrange("p (h e) -> p h e", h=H),
            bv_bc.rearrange("p (h e) -> p h e", h=H))

    def emit_proj(h):
        for ks in range(L // 512):
            pk = stpool.tile([128, LQ], F32, tag="st", name=f"pk{h}_{ks}")
            nc.tensor.matmul(
                pk[0:64, 0:512], wk_sb[:, h * D:(h + 1) * D],
                xT[:, ks * 512:(ks + 1) * 512],
                start=True, stop=True)
            nc.vector.tensor_scalar_add(
                kt[h][:, ks * 512:(ks + 1) * 512], pk[0:64, 0:512],
                bkT[:, h:h + 1])
        for qs in range(LQ // 512):
            pq = stpool.tile([128, LQ], F32, tag="st", name=f"pq{h}_{qs}")
            nc.tensor.matmul(
                pq[0:64, 0:512], wq_sb[:, h * D:(h + 1) * D],
                xT[:, qs * 512:(qs + 1) * 512],
                start=True, stop=True)
            nc.vector.tensor_scalar_add(
                qt[h][:, qs * 512:(qs + 1) * 512], pq[0:64, 0:512],
                bqT[:, h:h + 1])

    for lc in range(NKC):
        emit_v(lc)
    emit_proj(0)

    for h in range(H):
        j, half = h // 2, h % 2
        pv = pvpool.tile([65, LQ], F32, tag="pv")
        pts = {}

        def emit_pv(kc):
            for qs in range(LQ // 512):
                nc.tensor.matmul(
                    pv[:, qs * 512:(qs + 1) * 512],
                    v_view[:, kc, h, :],
                    pts[kc][:, qs * 512:(qs + 1) * 512],
                    start=(kc == 0), stop=(kc == NKC - 1))
            del pts[kc]

        for kc in range(NKC):
            st = stpool.tile([128, LQ], F32, tag="st")
            for qs in range(LQ // 512):
                nc.tensor.matmul(
                    st[:, qs * 512:(qs + 1) * 512],
                    kt[h][:, kc * 128:(kc + 1) * 128],
                    qt[h][:, qs * 512:(qs + 1) * 512],
                    start=True, stop=True)
            if kc > 0:
                emit_pv(kc - 1)
            pt = ptpool.tile([128, LQ], BF16, tag="pt")
            nc.scalar.activation(pt, st, mybir.ActivationFunctionType.Exp,
                                 bias=neg_c, scale=8.0)
            pts[kc] = pt
        if h + 1 < H:
            emit_proj(h + 1)   # fills the PE while exp(15) completes
        emit_pv(NKC - 1)

        # normalize: den row -> DRAM bounce -> [64,16] exact reciprocal ->
        # DRAM -> partition-broadcast [64, LQ]
        den_sb = denpool.tile([1, LQ], F32, tag="den_sb")
        nc.vector.tensor_copy(den_sb, pv[64:65, :])
        den_d = dendram.tile([1, LQ], F32, tag="den_d")
        nc.sync.dma_start(out=den_d, in_=den_sb)
        den64 = denpool.tile([64, 16], F32, tag="den64")
        nc.sync.dma_start(out=den64,
                          in_=den_d.rearrange("o (p i) -> (o p) i", p=64))
        rec64 = denpool.tile([64, 16], F32, tag="rec64")
        nc.vector.reciprocal(rec64, den64)
        rec_d = dendram.tile([1, LQ], F32, tag="rec_d")
        nc.sync.dma_start(out=rec_d.rearrange("o (p i) -> (o p) i", p=64),
                          in_=rec64)
        den_b = denpool.tile([64, LQ], F32, tag="den_b")
        nc.sync.dma_start(out=den_b, in_=rec_d.to_broadcast([64, LQ]))
        nc.vector.tensor_mul(
            ct[j][half * 64:(half + 1) * 64, :], pv[0:64, :], den_b)

    # ---------------- Phase D: output projection ----------------
    with tc.tile_pool(name="outs", bufs=3) as outsb:
        for lc in range(LQ // 128):
            pot = stpool.tile([128, LQ], F32, tag="st", name=f"po{lc}")
            po = pot[:, 0:D]
            for j in range(4):
                nc.tensor.matmul(
                    po, ct[j][:, lc * 128:(lc + 1) * 128], wo_sb[:, j, :],
                    start=(j == 0), stop=(j == 3))
            osb = outsb.tile([128, D], F32, tag="osb")
            nc.vector.tensor_add(osb, po, bo_bc)
            nc.sync.dma_start(out=o_d[lc * 128:(lc + 1) * 128, :], in_=osb)


def _build():
    if "nc" in _cache:
        return _cache["nc"]
    from contextlib import ExitStack
    nc = bacc.Bacc("TRN2", target_bir_lowering=False, debug=False)
    with tile.TileContext(nc) as tc:
        with ExitStack() as ctx:
            _emit(nc, tc, ctx)
    nc.finalize()
    _cache["nc"] = nc
    return nc


def kernel(x, wq, bq, wk, bk, wv, bv, wo, bo):
    global LAST_RESULTS
    x = np.ascontiguousarray(np.asarray(x, dtype=np.float32))
    wq = np.ascontiguousarray(np.asarray(wq, dtype=np.float32))
    wk = np.ascontiguousarray(np.asarray(wk, dtype=np.float32))
    wv = np.ascontiguousarray(np.asarray(wv, dtype=np.float32))
    wo = np.ascontiguousarray(np.asarray(wo, dtype=np.float32))
    bq = np.ascontiguousarray(np.asarray(bq, dtype=np.float32))
    bk = np.ascontiguousarray(np.asarray(bk, dtype=np.float32))
    bv = np.ascontiguousarray(np.asarray(bv, dtype=np.float32))
    bo = np.ascontiguousarray(np.asarray(bo, dtype=np.float32))

    # wo rows are dv-major (row = dv*H + h); permute to h-major (row = h*D+dv)
    wo_p = np.ascontiguousarray(
        wo.reshape(D, H, D).transpose(1, 0, 2).reshape(H * D, D))
    # QK/PV matmul operands are fed in 16-bit; convert weights on host
    wq16 = wq.astype(np.float16)
    wk16 = wk.astype(np.float16)
    wv16 = wv.astype(np.float16)
    wo16 = wo_p.astype(np.float16)

    nc = _build()
    in_maps = []
    for c in range(8):
        b, half = c // 2, c % 2
        xb = x[b]
        if half:
            xb = np.concatenate([xb[LQ:], xb[:LQ]], axis=0)
        xb = np.ascontiguousarray(xb.astype(np.float16))
        in_maps.append({
            "xin": xb, "wq": wq16, "wk": wk16, "wv": wv16, "wo_p": wo16,
            "bq": bq, "bk": bk, "bv": bv, "bo": bo,
        })
    res = run_bass_kernel_spmd(nc, in_maps, core_ids=list(range(8)),
                               trace=TRACE, tmpdir=TRACE_DIR)
    LAST_RESULTS = res
    out = np.empty((B, L, D), dtype=np.float32)
    for c in range(8):
        b, half = c // 2, c % 2
        out[b, half * LQ:(half + 1) * LQ, :] = res.results[c]["o"]
    return out
